# revision 15
# baseline (speedup 1.0000x reference)
#!/usr/bin/env python3
"""GroupedQueryAttention Trainium2 kernel, tensor-parallel over heads on 8
NeuronCores.

Reference model: B=2, S=2048, H=4096, NH=32 query heads, NKV=8 kv heads,
HD=128, RoPE base 5e5, softmax attention, o-proj.

Sharding: core c owns kv head c and query heads 4c..4c+3 (groups stay
aligned).  Wq/Wo sharded by query head, Wk/Wv by kv head.  Each core
computes a rank-512 slice of the o-proj contraction; the host sums the 8
partial outputs (bf16) in fp32.

On-core dataflow (per batch b):
  P: QKV projections.  X^T streamed h-major on the SP DMA queue; Wk/Wv
     streamed on the ACT queue; Wq resident.  One PSUM bank per output
     head; RoPE on DVE straight out of PSUM writes bf16 activations;
     V^T transposed to token-major via PE transposes (bf16).
  A+W fused: attention per 512-token chunk with the PREVIOUS chunk's
     o-proj matmuls interleaved into the PE stream, so the ACT-bound exp
     pipeline hides behind o-proj work.  Scores transposed (j on
     partitions); exp on ACT writes bf16 P; P@V streams per j-tile right
     behind exp; softmax denominator via bf16 pair-adds (DVE 2x) + Pool
     tree + ones-matmul; normalization by PE-broadcast reciprocal.
  Output partials written bf16, staged via Pool casts, DMA'd on the DVE
     queue.
"""
import sys

for _p in ("/opt/trn_rl_repo",):
    if _p not in sys.path:
        sys.path.insert(0, _p)

import numpy as np

import concourse.bacc as bacc
import concourse.mybir as mybir
from concourse import tile
from concourse.bass_utils import run_bass_kernel_spmd

B, S, H = 2, 2048, 4096
NH, NKV, HD = 32, 8, 128
NCORES = 8
QH = NH // NCORES                   # 4 query heads / core
QD = QH * HD                        # 512 q dims / core
ROPE_BASE = 500000.0
T = B * S                           # 4096 tokens
TCH = 512                           # projection token chunk
NTCH = S // TCH                     # 4 chunks per batch
ICH = 512                           # attention i-chunk
NICH = S // ICH
HT = H // 128                       # 32 h-tiles
JT = S // 128                       # 16 j-tiles per batch
NHCH = H // 512                     # 8 o-proj column chunks
SCALE = 1.0 / np.sqrt(HD)

F32 = mybir.dt.float32
F32R = mybir.dt.float32r
BF16 = mybir.dt.bfloat16


def _build_nc():
    nc = bacc.Bacc("TRN2", target_bir_lowering=False, debug=False)
    xt = nc.dram_tensor("xt", [H, T], F32R, kind="ExternalInput").ap()
    wq = nc.dram_tensor("wq", [H, QD], F32R, kind="ExternalInput").ap()
    wkv = nc.dram_tensor("wkv", [H, 256], F32R, kind="ExternalInput").ap()
    wob = nc.dram_tensor("wob", [QD, H], BF16, kind="ExternalInput").ap()
    cosx = nc.dram_tensor("cosx", [HD, S], F32, kind="ExternalInput").ap()
    ssin = nc.dram_tensor("ssin", [HD, S], F32, kind="ExternalInput").ap()
    identb = nc.dram_tensor("identb", [128, 128], BF16, kind="ExternalInput").ap()
    ones_bf = nc.dram_tensor("ones_bf", [128, 1], BF16, kind="ExternalInput").ap()
    ones_fr = nc.dram_tensor("ones_fr", [1, 128], F32R, kind="ExternalInput").ap()
    out = nc.dram_tensor("out_part", [T, H], BF16, kind="ExternalOutput").ap()

    with tile.TileContext(nc) as tc, \
         nc.allow_low_precision(reason="bf16 activations; fp32 accumulation"):
        with tc.tile_pool(name="persist", bufs=1) as persist, \
             tc.tile_pool(name="act", bufs=1) as apool:
            # residents.  Spread initial DMAs over queues; wq chunked so the
            # first projection matmuls can start early.
            wq_sb = persist.tile([128, HT, QD], F32R)
            wq_chunks = [(0, 2), (2, 8), (8, 16), (16, 24), (24, 32)]
            for c0, c1 in wq_chunks:
                nc.gpsimd.dma_start(
                    wq_sb[:, c0:c1, :],
                    wq[c0 * 128:c1 * 128, :]
                    .rearrange("(a p) q -> p a q", p=128))
            cos_sb = persist.tile([HD, S], F32)
            nc.scalar.dma_start(cos_sb[:], cosx[:])
            ssin_sb = persist.tile([HD, S], F32)
            nc.scalar.dma_start(ssin_sb[:], ssin[:])
            identb_sb = persist.tile([128, 128], BF16)
            nc.scalar.dma_start(identb_sb[:], identb[:])
            onesb_sb = persist.tile([128, 1], BF16)
            nc.scalar.dma_start(onesb_sb[:], ones_bf[:])
            onesr_sb = persist.tile([1, 128], F32R)
            nc.scalar.dma_start(onesr_sb[:], ones_fr[:])
            wo_sb = persist.tile([128, QH, H], BF16)
            nc.gpsimd.dma_start(
                wo_sb[:], wob.rearrange("(a p) hh -> p a hh", p=128))

            # cross-batch activation stores (tags shared across batches)
            qt_sb = [
                apool.tile([128, S], BF16, name=f"qt{qh}", tag=f"qt{qh}")
                for qh in range(QH)
            ]
            kt_sb = apool.tile([128, S], BF16, name="kt", tag="kt")
            v_sb = apool.tile([128, JT, 128], BF16, name="v", tag="v")

            def p_phase(b, px, pxw, ptmp, pps, vps):
                t0 = b * S
                for tch in range(NTCH):
                    tc0 = t0 + tch * TCH
                    q_ps = [
                        pps.tile([128, TCH], F32, name=f"qps{i}", tag=f"qps{i}")
                        for i in range(QH)
                    ]
                    k_ps = pps.tile([128, TCH], F32, name="kps", tag="kps")
                    v_ps = pps.tile([128, TCH], F32, name="vps0", tag="vps0")
                    for hg in range(HT // 2):
                        x_t = px.tile([128, 2, TCH], F32R, name="xs", tag="xs",
                                      bufs=3)
                        nc.sync.dma_start(
                            x_t[:],
                            xt[hg * 256:(hg + 1) * 256, tc0:tc0 + TCH]
                            .rearrange("(a p) t -> p a t", p=128))
                        wkv_t = pxw.tile([128, 2, 256], F32R, name="wkvs",
                                         tag="wkvs", bufs=3)
                        nc.scalar.dma_start(
                            wkv_t[:],
                            wkv[hg * 256:(hg + 1) * 256, :]
                            .rearrange("(a p) q -> p a q", p=128))
                        for hi in range(2):
                            h = hg * 2 + hi
                            first, last = h == 0, h == HT - 1
                            for qd in range(QH):
                                nc.tensor.matmul(
                                    q_ps[qd][:],
                                    wq_sb[:, h, qd * 128:(qd + 1) * 128],
                                    x_t[:, hi, :], start=first, stop=last)
                            nc.tensor.matmul(k_ps[:], wkv_t[:, hi, 0:128],
                                             x_t[:, hi, :], start=first,
                                             stop=last)
                            nc.tensor.matmul(v_ps[:], wkv_t[:, hi, 128:256],
                                             x_t[:, hi, :], start=first,
                                             stop=last)
                    # RoPE on DVE straight out of PSUM (mixed partition-base
                    # TT requires one PSUM operand on HW).  On the last tch,
                    # K goes first so attention starts sooner.
                    csl = slice(tch * TCH, (tch + 1) * TCH)
                    units = [(q_ps[qd], qt_sb[qd]) for qd in range(QH)]
                    units.append((k_ps, kt_sb))
                    if tch == NTCH - 1:
                        units = units[-1:] + units[:-1]
                    vraw = ptmp.tile([128, TCH], BF16, name="vraw", tag="vraw",
                                     bufs=2)
                    nc.scalar.copy(vraw[:], v_ps[:])

                    def rope(src, dst, dst_sl):
                        tA = ptmp.tile([128, TCH], F32, name="ropeA",
                                       tag="ropeA", bufs=2)
                        nc.vector.tensor_tensor(
                            tA[:], src[:], cos_sb[:, csl],
                            mybir.AluOpType.mult)
                        tB = ptmp.tile([128, TCH], F32, name="ropeB",
                                       tag="ropeB", bufs=2)
                        nc.vector.tensor_tensor(
                            tB[0:64, :], src[64:128, :],
                            ssin_sb[0:64, csl], mybir.AluOpType.mult)
                        nc.vector.tensor_tensor(
                            tB[64:128, :], src[0:64, :],
                            ssin_sb[64:128, csl], mybir.AluOpType.mult)
                        nc.vector.tensor_tensor(
                            dst[:, dst_sl], tA[:], tB[:], mybir.AluOpType.add)

                    for src, dst in units:
                        rope(src, dst, csl)
                    # V: PE-transpose bf16 staging to token-major
                    for tt in range(TCH // 128):
                        vt_ps = vps.tile([128, 128], BF16, name="vtp",
                                         tag="vtp")
                        nc.tensor.transpose(
                            vt_ps[:], vraw[:, tt * 128:(tt + 1) * 128],
                            identb_sb[:])
                        nc.scalar.copy(v_sb[:, tch * 4 + tt, :], vt_ps[:])

            def w_piece(pend, hch, wps, wstage):
                """o-proj for one 512-col slice of H over pending chunk."""
                pb, pich, pot = pend
                t0 = pb * S + pich * ICH
                ow = wstage.tile([128, 4, 512], BF16, name="ow", tag="ow",
                                 bufs=2)
                for tt in range(4):
                    w_ps = wps.tile([128, 512], F32, name="wops", tag="wops")
                    for od in range(QH):
                        nc.tensor.matmul(
                            w_ps[:],
                            pot[od][:, tt * 128:(tt + 1) * 128],
                            wo_sb[:, od, hch * 512:(hch + 1) * 512],
                            start=(od == 0), stop=(od == QH - 1))
                    nc.vector.tensor_copy(ow[:, tt, :], w_ps[:])
                nc.sync.dma_start(
                    out[t0:t0 + ICH, hch * 512:(hch + 1) * 512]
                    .rearrange("(a p) hh -> p a hh", p=128),
                    ow[:])

            def a_head(b, ich, qh, ot_cur, pend, ap, tpool, norm, sps, ops_,
                       dps, bcps, wps, wstage):
                isl = slice(ich * ICH, (ich + 1) * ICH)
                p_t = [None] * JT
                o_ps = ops_.tile([128, ICH], F32, name="ops0", tag="ops0")
                t8 = tpool.tile([128, 8, ICH], BF16, name="t8", tag="t8",
                                bufs=1)

                def scores(jt):
                    st_ps = sps.tile([128, ICH], F32, name="st", tag="st")
                    nc.tensor.matmul(
                        st_ps[:], kt_sb[:, jt * 128:(jt + 1) * 128],
                        qt_sb[qh][:, isl], start=True, stop=True)
                    p_t[jt] = ap.tile([128, ICH], BF16, name="ptil",
                                      tag="ptil", bufs=4)
                    nc.scalar.activation(
                        p_t[jt][:], st_ps[:],
                        mybir.ActivationFunctionType.Exp, scale=SCALE)
                    if jt % 2 == 1:
                        nc.gpsimd.tensor_tensor(
                            t8[:, jt // 2, :], p_t[jt - 1][:], p_t[jt][:],
                            mybir.AluOpType.add)

                def pv(jt):
                    nc.tensor.matmul(o_ps[:], v_sb[:, jt, :], p_t[jt][:],
                                     start=(jt == 0), stop=(jt == JT - 1))

                # PE stream: scores run 5 ahead of PV; two o-proj slices of
                # the pending chunk inserted mid-head; den/bc before PV tail.
                for jt in range(5):
                    scores(jt)
                for jt in range(5, 10):
                    pv(jt - 5)
                    scores(jt)
                if pend is not None:
                    w_piece(pend, 2 * qh, wps, wstage)
                for jt in range(10, 12):
                    pv(jt - 5)
                    scores(jt)
                if pend is not None:
                    w_piece(pend, 2 * qh + 1, wps, wstage)
                for jt in range(12, 16):
                    pv(jt - 5)
                    scores(jt)
                pv(11)
                # denominator: bf16 pair-tree tail on Pool, partition-sum on PE
                t4 = tpool.tile([128, 4, ICH], BF16, name="t4", tag="t4",
                                bufs=1)
                nc.gpsimd.tensor_tensor(t4[:], t8[:, 0:4, :], t8[:, 4:8, :],
                                        mybir.AluOpType.add)
                t2 = tpool.tile([128, 2, ICH], BF16, name="t2", tag="t2",
                                bufs=1)
                nc.gpsimd.tensor_tensor(t2[:], t4[:, 0:2, :], t4[:, 2:4, :],
                                        mybir.AluOpType.add)
                t1 = tpool.tile([128, ICH], BF16, name="t1", tag="t1", bufs=1)
                nc.gpsimd.tensor_tensor(t1[:], t2[:, 0, :], t2[:, 1, :],
                                        mybir.AluOpType.add)
                den_ps = dps.tile([1, ICH], F32, name="den", tag="den")
                nc.tensor.matmul(den_ps[:], onesb_sb[:], t1[:],
                                 start=True, stop=True)
                rec = norm.tile([1, ICH], F32R, name="rec", tag="rec", bufs=2)
                nc.vector.reciprocal(rec[:], den_ps[:])
                bc_ps = bcps.tile([128, ICH], F32, name="bc", tag="bc")
                nc.tensor.matmul(bc_ps[:], onesr_sb[:], rec[:],
                                 start=True, stop=True)
                bc_sb = norm.tile([128, ICH], BF16, name="bcs", tag="bcs",
                                  bufs=2)
                nc.vector.tensor_copy(bc_sb[:], bc_ps[:])
                for jt in range(12, 16):
                    pv(jt)
                nc.vector.tensor_tensor(ot_cur[qh][:], o_ps[:], bc_sb[:],
                                        mybir.AluOpType.mult)

            pend = None
            for b in range(B):
                with tc.tile_pool(name="px", bufs=1) as px, \
                     tc.tile_pool(name="pxw", bufs=1) as pxw, \
                     tc.tile_pool(name="pt", bufs=1) as ptmp, \
                     tc.tile_pool(name="pps", bufs=1, space="PSUM") as pps, \
                     tc.tile_pool(name="vps", bufs=2, space="PSUM") as vps:
                    p_phase(b, px, pxw, ptmp, pps, vps)
                with tc.tile_pool(name="ap", bufs=1) as ap, \
                     tc.tile_pool(name="tp", bufs=1) as tpool, \
                     tc.tile_pool(name="nrm", bufs=1) as norm, \
                     tc.tile_pool(name="wst", bufs=1) as wstage, \
                     tc.tile_pool(name="sps", bufs=2, space="PSUM") as sps, \
                     tc.tile_pool(name="ops", bufs=2, space="PSUM") as ops_, \
                     tc.tile_pool(name="dps", bufs=1, space="PSUM") as dps, \
                     tc.tile_pool(name="bcp", bufs=1, space="PSUM") as bcps, \
                     tc.tile_pool(name="wps", bufs=2, space="PSUM") as wps:
                    for ich in range(NICH):
                        ot_cur = [
                            apool.tile([128, ICH], BF16, name=f"ot{qh}",
                                       tag=f"ot{qh}", bufs=2)
                            for qh in range(QH)
                        ]
                        for qh in range(QH):
                            a_head(b, ich, qh, ot_cur, pend, ap, tpool, norm,
                                   sps, ops_, dps, bcps, wps, wstage)
                        pend = (b, ich, ot_cur)
                    if b == B - 1:
                        # final chunk's o-proj
                        for hch in range(NHCH):
                            w_piece(pend, hch, wps, wstage)
    nc.finalize()
    return nc


_NC_CACHE = None


def _get_nc():
    global _NC_CACHE
    if _NC_CACHE is None:
        _NC_CACHE = _build_nc()
    return _NC_CACHE


def _host_tables():
    inv = 1.0 / (ROPE_BASE ** (np.arange(0, HD, 2, dtype=np.float64) / HD))
    t = np.arange(S, dtype=np.float64)
    freqs = np.outer(t, inv)                      # [S, 64]
    emb = np.concatenate([freqs, freqs], axis=1)  # [S, 128]
    cos = np.cos(emb).astype(np.float32).T.copy()   # [128, S]
    sin = np.sin(emb).astype(np.float32).T.copy()
    ssin = sin.copy()
    ssin[0:64, :] *= -1.0
    return np.ascontiguousarray(cos), np.ascontiguousarray(ssin)


def kernel(hidden_states, Wq, Wk, Wv, Wo, trace=False):
    import ml_dtypes
    hs = np.asarray(hidden_states, dtype=np.float32)
    Wq = np.asarray(Wq, dtype=np.float32)
    Wk = np.asarray(Wk, dtype=np.float32)
    Wv = np.asarray(Wv, dtype=np.float32)
    Wo = np.asarray(Wo, dtype=np.float32)

    xt = np.ascontiguousarray(hs.reshape(T, H).T)          # [H, T]
    cos, ssin = _host_tables()
    identb = np.eye(128, dtype=ml_dtypes.bfloat16)
    ones_bf = np.ones((128, 1), dtype=ml_dtypes.bfloat16)
    ones_fr = np.ones((1, 128), dtype=np.float32)

    in_maps = []
    for c in range(NCORES):
        wkv = np.concatenate(
            [Wk[c * HD:(c + 1) * HD, :].T, Wv[c * HD:(c + 1) * HD, :].T],
            axis=1)                                        # [H, 256]
        in_maps.append({
            "xt": xt,
            "wq": np.ascontiguousarray(Wq[c * QD:(c + 1) * QD, :].T),
            "wkv": np.ascontiguousarray(wkv),
            "wob": np.ascontiguousarray(
                Wo[:, c * QD:(c + 1) * QD].T).astype(ml_dtypes.bfloat16),
            "cosx": cos,
            "ssin": ssin,
            "identb": identb,
            "ones_bf": ones_bf,
            "ones_fr": ones_fr,
        })

    nc = _get_nc()
    res = run_bass_kernel_spmd(nc, in_maps, list(range(NCORES)), trace=trace)
    acc = np.zeros((T, H), dtype=np.float32)
    for c in range(NCORES):
        acc += res.results[c]["out_part"].astype(np.float32)
    out = acc.reshape(B, S, H)
    if trace:
        return out, res
    return out


# revision 28
# speedup vs baseline: 1.0513x; 1.0513x over previous
#!/usr/bin/env python3
"""GroupedQueryAttention Trainium2 kernel, tensor-parallel over heads on 8
NeuronCores.

Reference model: B=2, S=2048, H=4096, NH=32 query heads, NKV=8 kv heads,
HD=128, RoPE base 5e5, softmax attention, o-proj.

Sharding: core c owns kv head c and query heads 4c..4c+3 (groups stay
aligned).  Wq/Wo sharded by query head, Wk/Wv by kv head.  Each core
computes a rank-512 slice of the o-proj contraction; the host sums the 8
partial outputs (bf16) in fp32.

On-core dataflow (per batch b):
  P: QKV projections.  X^T streamed h-major on the SP DMA queue; Wk/Wv
     streamed on the ACT queue; Wq resident.  One PSUM bank per output
     head; PSUM drains fast through straight (DVE) + partition-swapped
     (ACT) staging copies, then RoPE runs SBUF->SBUF on DVE writing bf16
     per-chunk activation tiles (per-chunk so attention only depends on
     the chunk that feeds it); V^T transposed token-major by PE (bf16).
  A+W fused: attention per 512-token chunk with the PREVIOUS chunk's
     o-proj matmuls interleaved into the PE stream, so the ACT-bound exp
     pipeline hides behind o-proj work.  Scores transposed (j on
     partitions); exp on ACT writes bf16 P per j-tile; P@V streams right
     behind exp; softmax denominator via bf16 pair-adds + Pool tree +
     ones-matmul; normalization by PE-broadcast reciprocal.  o-proj
     PSUM lives on the right side of the bank space so it never aliases
     projection banks across the phase boundary.
  Output partials written bf16 (DVE casts), DMA'd on the SP queue.

HW pitfalls baked in (walrus BIR verifier / observed miscompiles):
  - GPSIMD cannot touch PSUM; all PSUM reads go through ACT/DVE.
  - tensor_tensor with both inputs in SBUF requires equal base
    partitions; rotate-half is staged via partition-shifted copies
    (legal on ACT/DVE) instead.
  - dma_start_transpose produces wrong data when composed with an
    ACT-copy producer (unsynced on HW); V uses PE transposes.
"""
import sys

for _p in ("/opt/trn_rl_repo",):
    if _p not in sys.path:
        sys.path.insert(0, _p)

import numpy as np

import concourse.bacc as bacc
import concourse.mybir as mybir
from concourse import tile
from concourse.bass_utils import run_bass_kernel_spmd

B, S, H = 2, 2048, 4096
NH, NKV, HD = 32, 8, 128
NCORES = 8
QH = NH // NCORES                   # 4 query heads / core
QD = QH * HD                        # 512 q dims / core
ROPE_BASE = 500000.0
T = B * S                           # 4096 tokens
TCH = 512                           # projection token chunk
NTCH = S // TCH                     # 4 chunks per batch
ICH = 512                           # attention i-chunk
NICH = S // ICH
HT = H // 128                       # 32 h-tiles
JT = S // 128                       # 16 j-tiles per batch
NHCH = H // 512                     # 8 o-proj column chunks
SCALE = 1.0 / np.sqrt(HD)

F32 = mybir.dt.float32
F32R = mybir.dt.float32r
BF16 = mybir.dt.bfloat16


def _build_nc():
    nc = bacc.Bacc("TRN2", target_bir_lowering=False, debug=False)
    xt = nc.dram_tensor("xt", [H, T], F32R, kind="ExternalInput").ap()
    wq = nc.dram_tensor("wq", [H, QD], F32R, kind="ExternalInput").ap()
    wkv = nc.dram_tensor("wkv", [H, 256], F32R, kind="ExternalInput").ap()
    wob = nc.dram_tensor("wob", [QD, H], BF16, kind="ExternalInput").ap()
    cosx = nc.dram_tensor("cosx", [HD, S], F32, kind="ExternalInput").ap()
    ssin = nc.dram_tensor("ssin", [HD, S], F32, kind="ExternalInput").ap()
    identb = nc.dram_tensor("identb", [128, 128], BF16, kind="ExternalInput").ap()
    ones_bf = nc.dram_tensor("ones_bf", [128, 1], BF16, kind="ExternalInput").ap()
    ones_fr = nc.dram_tensor("ones_fr", [1, 128], F32R, kind="ExternalInput").ap()
    out = nc.dram_tensor("out_part", [T, H], BF16, kind="ExternalOutput").ap()

    with tile.TileContext(nc) as tc, \
         nc.allow_low_precision(reason="bf16 activations; fp32 accumulation"):
        with tc.tile_pool(name="persist", bufs=1) as persist, \
             tc.tile_pool(name="act", bufs=1) as apool:
            # residents.  Spread initial DMAs over queues; wq chunked so the
            # first projection matmuls can start early.
            wq_sb = persist.tile([128, HT, QD], F32R)
            wq_chunks = [(0, 2), (2, 8), (8, 16), (16, 24), (24, 32)]
            for c0, c1 in wq_chunks:
                nc.gpsimd.dma_start(
                    wq_sb[:, c0:c1, :],
                    wq[c0 * 128:c1 * 128, :]
                    .rearrange("(a p) q -> p a q", p=128))
            cos_sb = persist.tile([HD, S], F32)
            nc.scalar.dma_start(cos_sb[:], cosx[:])
            ssin_sb = persist.tile([HD, S], F32)
            nc.scalar.dma_start(ssin_sb[:], ssin[:])
            identb_sb = persist.tile([128, 128], BF16)
            nc.scalar.dma_start(identb_sb[:], identb[:])
            onesb_sb = persist.tile([128, 1], BF16)
            nc.scalar.dma_start(onesb_sb[:], ones_bf[:])
            onesr_sb = persist.tile([1, 128], F32R)
            nc.scalar.dma_start(onesr_sb[:], ones_fr[:])
            wo_sb = persist.tile([128, QH, H], BF16)
            nc.gpsimd.dma_start(
                wo_sb[:], wob.rearrange("(a p) hh -> p a hh", p=128))

            # cross-batch activation stores (tags shared across batches).
            # qt/kt are per-tch tiles so attention chunks depend only on the
            # projection chunk that produced them.
            qt_t = [
                [apool.tile([128, TCH], BF16, name=f"qt{qh}t{t}",
                            tag=f"qt{qh}t{t}") for t in range(NTCH)]
                for qh in range(QH)
            ]
            kt_t = [
                apool.tile([128, TCH], BF16, name=f"kt{t}", tag=f"kt{t}")
                for t in range(NTCH)
            ]
            v_t = [
                apool.tile([128, 4, 128], BF16, name=f"v{t}", tag=f"v{t}")
                for t in range(NTCH)
            ]

            def p_phase(b, px, pxw, pps, vps):
                t0 = b * S
                for tch in range(NTCH):
                    tc0 = t0 + tch * TCH
                    q_ps = [
                        pps.tile([128, TCH], F32, name=f"qps{i}", tag=f"qps{i}")
                        for i in range(QH)
                    ]
                    k_ps = pps.tile([128, TCH], F32, name="kps", tag="kps")
                    v_ps = pps.tile([128, TCH], F32, name="vps0", tag="vps0")
                    for hg in range(HT // 2):
                        x_t = px.tile([128, 2, TCH], F32R, name="xs", tag="xs",
                                      bufs=3)
                        nc.sync.dma_start(
                            x_t[:],
                            xt[hg * 256:(hg + 1) * 256, tc0:tc0 + TCH]
                            .rearrange("(a p) t -> p a t", p=128))
                        wkv_t = pxw.tile([128, 2, 256], F32R, name="wkvs",
                                         tag="wkvs", bufs=3)
                        nc.scalar.dma_start(
                            wkv_t[:],
                            wkv[hg * 256:(hg + 1) * 256, :]
                            .rearrange("(a p) q -> p a q", p=128))
                        for hi in range(2):
                            h = hg * 2 + hi
                            first, last = h == 0, h == HT - 1
                            for qd in range(QH):
                                nc.tensor.matmul(
                                    q_ps[qd][:],
                                    wq_sb[:, h, qd * 128:(qd + 1) * 128],
                                    x_t[:, hi, :], start=first, stop=last)
                            nc.tensor.matmul(k_ps[:], wkv_t[:, hi, 0:128],
                                             x_t[:, hi, :], start=first,
                                             stop=last)
                            nc.tensor.matmul(v_ps[:], wkv_t[:, hi, 128:256],
                                             x_t[:, hi, :], start=first,
                                             stop=last)
                    # RoPE: rotate-half staged via partition-shifted ACT
                    # copies (PSUM frees after one copy pair + the cos mult),
                    # then full-width DVE ops.  tA muls issued first so PSUM
                    # banks release at DVE pace for the next chunk.  On the
                    # last tch, K goes first so attention starts sooner.
                    csl = slice(tch * TCH, (tch + 1) * TCH)
                    units = [(q_ps[qd], qt_t[qd][tch], slice(0, TCH))
                             for qd in range(QH)]
                    units.append((k_ps, kt_t[tch], slice(0, TCH)))
                    if tch == NTCH - 1:
                        units = units[-1:] + units[:-1]
                    vraw = apool.tile([128, TCH], BF16, name="vraw",
                                      tag="vraw", bufs=2)
                    nc.scalar.copy(vraw[:], v_ps[:])
                    sqs, sws = [], []
                    for u, (src, dst, dsl) in enumerate(units):
                        sq = apool.tile([128, TCH], F32, name=f"sq{u}",
                                        tag=f"sq{u}", bufs=1)
                        nc.vector.tensor_copy(sq[:], src[:])
                        sqs.append(sq)
                        sw = apool.tile([128, TCH], F32, name=f"sw{u}",
                                        tag=f"sw{u}", bufs=1)
                        nc.scalar.copy(sw[0:64, :], src[64:128, :])
                        nc.scalar.copy(sw[64:128, :], src[0:64, :])
                        sws.append(sw)
                    for u, (src, dst, dsl) in enumerate(units):
                        tA = apool.tile([128, TCH], F32, name="ropeA",
                                        tag="ropeA", bufs=2)
                        nc.vector.tensor_tensor(
                            tA[:], sqs[u][:], cos_sb[:, csl],
                            mybir.AluOpType.mult)
                        tB = apool.tile([128, TCH], F32, name="ropeB",
                                        tag="ropeB", bufs=2)
                        nc.vector.tensor_tensor(
                            tB[:], sws[u][:], ssin_sb[:, csl],
                            mybir.AluOpType.mult)
                        nc.vector.tensor_tensor(
                            dst[:, dsl], tA[:], tB[:],
                            mybir.AluOpType.add)
                    # V: PE-transpose bf16 staging to token-major (the XBAR
                    # DMA transpose races with its ACT-copy producer on HW)
                    for tt in range(TCH // 128):
                        vt_ps = vps.tile([128, 128], BF16, name="vtp",
                                         tag="vtp")
                        nc.tensor.transpose(
                            vt_ps[:], vraw[:, tt * 128:(tt + 1) * 128],
                            identb_sb[:])
                        nc.scalar.copy(v_t[tch][:, tt, :], vt_ps[:])

            def w_piece(pend, hch, wps, wstage):
                """o-proj for one 512-col slice of H over pending chunk."""
                pb, pich, pot = pend
                t0 = pb * S + pich * ICH
                ow = wstage.tile([128, 4, 512], BF16, name="ow", tag="ow",
                                 bufs=2)
                for tt in range(4):
                    w_ps = wps.tile([128, 512], F32, name="wops", tag="wops")
                    for od in range(QH):
                        nc.tensor.matmul(
                            w_ps[:],
                            pot[od][:, tt * 128:(tt + 1) * 128],
                            wo_sb[:, od, hch * 512:(hch + 1) * 512],
                            start=(od == 0), stop=(od == QH - 1))
                    nc.vector.tensor_copy(ow[:, tt, :], w_ps[:])
                nc.sync.dma_start(
                    out[t0:t0 + ICH, hch * 512:(hch + 1) * 512]
                    .rearrange("(a p) hh -> p a hh", p=128),
                    ow[:])

            def a_head(b, ich, qh, ot_cur, pend, ap, tpool, norm, sps, ops_,
                       dps, bcps, wps, wstage):
                isl = slice(ich * ICH, (ich + 1) * ICH)
                p_t = [None] * JT
                o_ps = ops_.tile([128, ICH], F32, name="ops0", tag="ops0")
                t8 = tpool.tile([128, 8, ICH], BF16, name="t8", tag="t8",
                                bufs=1)

                def scores(jt):
                    st_ps = sps.tile([128, ICH], F32, name="st", tag="st")
                    nc.tensor.matmul(
                        st_ps[:],
                        kt_t[jt // 4][:, (jt % 4) * 128:(jt % 4 + 1) * 128],
                        qt_t[qh][ich][:], start=True, stop=True)
                    p_t[jt] = ap.tile([128, ICH], BF16, name="ptil",
                                      tag="ptil", bufs=4)
                    nc.scalar.activation(
                        p_t[jt][:], st_ps[:],
                        mybir.ActivationFunctionType.Exp, scale=SCALE)
                    if jt % 2 == 1:
                        nc.gpsimd.tensor_tensor(
                            t8[:, jt // 2, :], p_t[jt - 1][:], p_t[jt][:],
                            mybir.AluOpType.add)

                def pv(jt):
                    nc.tensor.matmul(o_ps[:], v_t[jt // 4][:, jt % 4, :],
                                     p_t[jt][:],
                                     start=(jt == 0), stop=(jt == JT - 1))

                # PE stream: scores run 5 ahead of PV; two o-proj slices of
                # the pending chunk inserted per head (at the head start for
                # qh0, where the boundary stall lives); den/bc before PV tail.
                if pend is not None and qh == 0:
                    w_piece(pend, 0, wps, wstage)
                for jt in range(5):
                    scores(jt)
                for jt in range(5, 10):
                    pv(jt - 5)
                    scores(jt)
                if pend is not None and qh != 0:
                    w_piece(pend, 2 * qh, wps, wstage)
                for jt in range(10, 12):
                    pv(jt - 5)
                    scores(jt)
                if pend is not None:
                    w_piece(pend, 2 * qh + 1, wps, wstage)
                for jt in range(12, 16):
                    pv(jt - 5)
                    scores(jt)
                pv(11)
                # denominator: bf16 pair-tree tail on Pool, partition-sum on PE
                t4 = tpool.tile([128, 4, ICH], BF16, name="t4", tag="t4",
                                bufs=1)
                nc.gpsimd.tensor_tensor(t4[:], t8[:, 0:4, :], t8[:, 4:8, :],
                                        mybir.AluOpType.add)
                t2 = tpool.tile([128, 2, ICH], BF16, name="t2", tag="t2",
                                bufs=1)
                nc.gpsimd.tensor_tensor(t2[:], t4[:, 0:2, :], t4[:, 2:4, :],
                                        mybir.AluOpType.add)
                t1 = tpool.tile([128, ICH], BF16, name="t1", tag="t1", bufs=1)
                nc.gpsimd.tensor_tensor(t1[:], t2[:, 0, :], t2[:, 1, :],
                                        mybir.AluOpType.add)
                den_ps = dps.tile([1, ICH], F32, name="den", tag="den")
                nc.tensor.matmul(den_ps[:], onesb_sb[:], t1[:],
                                 start=True, stop=True)
                rec = norm.tile([1, ICH], F32R, name="rec", tag="rec", bufs=2)
                nc.vector.reciprocal(rec[:], den_ps[:])
                bc_ps = bcps.tile([128, ICH], F32, name="bc", tag="bc")
                nc.tensor.matmul(bc_ps[:], onesr_sb[:], rec[:],
                                 start=True, stop=True)
                bc_sb = norm.tile([128, ICH], BF16, name="bcs", tag="bcs",
                                  bufs=2)
                nc.vector.tensor_copy(bc_sb[:], bc_ps[:])
                for jt in range(12, 16):
                    pv(jt)
                nc.vector.tensor_tensor(ot_cur[qh][:], o_ps[:], bc_sb[:],
                                        mybir.AluOpType.mult)

            pend = None
            for b in range(B):
                with tc.tile_pool(name="px", bufs=1) as px, \
                     tc.tile_pool(name="pxw", bufs=1) as pxw, \
                     tc.tile_pool(name="pps", bufs=1, space="PSUM") as pps, \
                     tc.tile_pool(name="vps", bufs=2, space="PSUM",
                                  side="right") as vps:
                    p_phase(b, px, pxw, pps, vps)
                with tc.tile_pool(name="ap", bufs=1) as ap, \
                     tc.tile_pool(name="tp", bufs=1) as tpool, \
                     tc.tile_pool(name="nrm", bufs=1) as norm, \
                     tc.tile_pool(name="wst", bufs=1) as wstage, \
                     tc.tile_pool(name="wps", bufs=2, space="PSUM",
                                  side="right") as wps, \
                     tc.tile_pool(name="sps", bufs=2, space="PSUM") as sps, \
                     tc.tile_pool(name="ops", bufs=2, space="PSUM") as ops_, \
                     tc.tile_pool(name="dps", bufs=1, space="PSUM") as dps, \
                     tc.tile_pool(name="bcp", bufs=1, space="PSUM") as bcps:
                    for ich in range(NICH):
                        ot_cur = [
                            apool.tile([128, ICH], BF16, name=f"ot{qh}",
                                       tag=f"ot{qh}", bufs=2)
                            for qh in range(QH)
                        ]
                        for qh in range(QH):
                            a_head(b, ich, qh, ot_cur, pend, ap, tpool, norm,
                                   sps, ops_, dps, bcps, wps, wstage)
                        pend = (b, ich, ot_cur)
                    if b == B - 1:
                        # final chunk's o-proj
                        for hch in range(NHCH):
                            w_piece(pend, hch, wps, wstage)
    nc.finalize()
    return nc


_NC_CACHE = None


def _get_nc():
    global _NC_CACHE
    if _NC_CACHE is None:
        _NC_CACHE = _build_nc()
    return _NC_CACHE


def _host_tables():
    inv = 1.0 / (ROPE_BASE ** (np.arange(0, HD, 2, dtype=np.float64) / HD))
    t = np.arange(S, dtype=np.float64)
    freqs = np.outer(t, inv)                      # [S, 64]
    emb = np.concatenate([freqs, freqs], axis=1)  # [S, 128]
    cos = np.cos(emb).astype(np.float32).T.copy()   # [128, S]
    sin = np.sin(emb).astype(np.float32).T.copy()
    ssin = sin.copy()
    ssin[0:64, :] *= -1.0
    return np.ascontiguousarray(cos), np.ascontiguousarray(ssin)


def kernel(hidden_states, Wq, Wk, Wv, Wo, trace=False):
    import ml_dtypes
    hs = np.asarray(hidden_states, dtype=np.float32)
    Wq = np.asarray(Wq, dtype=np.float32)
    Wk = np.asarray(Wk, dtype=np.float32)
    Wv = np.asarray(Wv, dtype=np.float32)
    Wo = np.asarray(Wo, dtype=np.float32)

    xt = np.ascontiguousarray(hs.reshape(T, H).T)          # [H, T]
    cos, ssin = _host_tables()
    ones_bf = np.ones((128, 1), dtype=ml_dtypes.bfloat16)
    ones_fr = np.ones((1, 128), dtype=np.float32)

    in_maps = []
    for c in range(NCORES):
        wkv = np.concatenate(
            [Wk[c * HD:(c + 1) * HD, :].T, Wv[c * HD:(c + 1) * HD, :].T],
            axis=1)                                        # [H, 256]
        in_maps.append({
            "xt": xt,
            "wq": np.ascontiguousarray(Wq[c * QD:(c + 1) * QD, :].T),
            "wkv": np.ascontiguousarray(wkv),
            "wob": np.ascontiguousarray(
                Wo[:, c * QD:(c + 1) * QD].T).astype(ml_dtypes.bfloat16),
            "identb": np.eye(128, dtype=ml_dtypes.bfloat16),
            "cosx": cos,
            "ssin": ssin,
            "ones_bf": ones_bf,
            "ones_fr": ones_fr,
        })

    nc = _get_nc()
    res = run_bass_kernel_spmd(nc, in_maps, list(range(NCORES)), trace=trace)
    acc = np.zeros((T, H), dtype=np.float32)
    for c in range(NCORES):
        acc += res.results[c]["out_part"].astype(np.float32)
    out = acc.reshape(B, S, H)
    if trace:
        return out, res
    return out


# revision 38
# speedup vs baseline: 1.0625x; 1.0106x over previous
#!/usr/bin/env python3
"""GroupedQueryAttention Trainium2 kernel, tensor-parallel over heads on 8
NeuronCores.

Reference model: B=2, S=2048, H=4096, NH=32 query heads, NKV=8 kv heads,
HD=128, RoPE base 5e5, softmax attention, o-proj.

Sharding: core c owns kv head c and query heads 4c..4c+3 (groups stay
aligned).  Wq/Wo sharded by query head, Wk/Wv by kv head.  Each core
computes a rank-512 slice of the o-proj contraction; the host sums the 8
partial outputs (bf16) in fp32.

On-core dataflow (per batch b):
  P: QKV projections.  X^T streamed h-major on the SP DMA queue; Wk/Wv
     streamed on the ACT queue; Wq resident.  One PSUM bank per output
     head; PSUM drains fast through straight (DVE) + partition-swapped
     (ACT) staging copies, then RoPE runs SBUF->SBUF on DVE writing bf16
     per-chunk activation tiles (per-chunk so attention only depends on
     the chunk that feeds it); V^T transposed token-major by PE (bf16).
  A+W fused: attention per 512-token chunk with the PREVIOUS chunk's
     o-proj matmuls interleaved into the PE stream, so the ACT-bound exp
     pipeline hides behind o-proj work.  Scores transposed (j on
     partitions); one exp per j-tile PAIR (halves ACT instruction
     overhead); P@V streams right behind exp; softmax denominator via
     bf16 pair-adds + Pool tree + a single all-ones 128x128 matmul that
     sums partitions AND broadcasts in one shot; reciprocal lands bf16
     straight into SBUF.  o-proj PSUM lives on the right side of the
     bank space so it never aliases projection banks across the phase
     boundary; two o-proj slices are emitted at the batch boundary to
     fill the softmax-chain drain.
  Output partials written bf16 (DVE casts), DMA'd on the SP queue.

HW pitfalls baked in (walrus BIR verifier / observed miscompiles):
  - GPSIMD cannot touch PSUM; all PSUM reads go through ACT/DVE.
  - tensor_tensor with both inputs in SBUF requires equal base
    partitions; rotate-half is staged via partition-shifted copies
    (legal on ACT/DVE) instead.
  - dma_start_transpose produces wrong data when composed with an
    ACT-copy producer (unsynced on HW); V uses PE transposes.
"""
import sys

for _p in ("/opt/trn_rl_repo",):
    if _p not in sys.path:
        sys.path.insert(0, _p)

import numpy as np

import concourse.bacc as bacc
import concourse.mybir as mybir
from concourse import tile
from concourse.bass_utils import run_bass_kernel_spmd

B, S, H = 2, 2048, 4096
NH, NKV, HD = 32, 8, 128
NCORES = 8
QH = NH // NCORES                   # 4 query heads / core
QD = QH * HD                        # 512 q dims / core
ROPE_BASE = 500000.0
T = B * S                           # 4096 tokens
TCH = 512                           # projection token chunk
NTCH = S // TCH                     # 4 chunks per batch
ICH = 512                           # attention i-chunk
NICH = S // ICH
HT = H // 128                       # 32 h-tiles
JT = S // 128                       # 16 j-tiles per batch
NHCH = H // 512                     # 8 o-proj column chunks
SCALE = 1.0 / np.sqrt(HD)

F32 = mybir.dt.float32
F32R = mybir.dt.float32r
BF16 = mybir.dt.bfloat16


def _build_nc():
    nc = bacc.Bacc("TRN2", target_bir_lowering=False, debug=False)
    xt = nc.dram_tensor("xt", [H, T], F32R, kind="ExternalInput").ap()
    wq = nc.dram_tensor("wq", [H, QD], F32R, kind="ExternalInput").ap()
    wkv = nc.dram_tensor("wkv", [H, 256], F32R, kind="ExternalInput").ap()
    wob = nc.dram_tensor("wob", [QD, H], BF16, kind="ExternalInput").ap()
    cosx = nc.dram_tensor("cosx", [HD, S], F32, kind="ExternalInput").ap()
    ssin = nc.dram_tensor("ssin", [HD, S], F32, kind="ExternalInput").ap()
    identb = nc.dram_tensor("identb", [128, 128], BF16, kind="ExternalInput").ap()
    ones_sq = nc.dram_tensor("ones_sq", [128, 128], BF16, kind="ExternalInput").ap()
    out = nc.dram_tensor("out_part", [T, H], BF16, kind="ExternalOutput").ap()

    with tile.TileContext(nc) as tc, \
         nc.allow_low_precision(reason="bf16 activations; fp32 accumulation"):
        with tc.tile_pool(name="persist", bufs=1) as persist, \
             tc.tile_pool(name="act", bufs=1) as apool:
            # residents.  Spread initial DMAs over queues; wq chunked so the
            # first projection matmuls can start early.
            wq_sb = persist.tile([128, HT, QD], F32R)
            wq_chunks = [(0, 2), (2, 8), (8, 16), (16, 24), (24, 32)]
            for c0, c1 in wq_chunks:
                nc.gpsimd.dma_start(
                    wq_sb[:, c0:c1, :],
                    wq[c0 * 128:c1 * 128, :]
                    .rearrange("(a p) q -> p a q", p=128))
            cos_sb = persist.tile([HD, S], F32)
            nc.scalar.dma_start(cos_sb[:], cosx[:])
            ssin_sb = persist.tile([HD, S], F32)
            nc.scalar.dma_start(ssin_sb[:], ssin[:])
            identb_sb = persist.tile([128, 128], BF16)
            nc.scalar.dma_start(identb_sb[:], identb[:])
            onesq_sb = persist.tile([128, 128], BF16)
            nc.scalar.dma_start(onesq_sb[:], ones_sq[:])
            wo_sb = persist.tile([128, QH, H], BF16)
            nc.gpsimd.dma_start(
                wo_sb[:], wob.rearrange("(a p) hh -> p a hh", p=128))

            # cross-batch activation stores (tags shared across batches).
            # qt/kt are per-tch tiles so attention chunks depend only on the
            # projection chunk that produced them.
            qt_t = [
                [apool.tile([128, TCH], BF16, name=f"qt{qh}t{t}",
                            tag=f"qt{qh}t{t}") for t in range(NTCH)]
                for qh in range(QH)
            ]
            kt_t = [
                apool.tile([128, TCH], BF16, name=f"kt{t}", tag=f"kt{t}")
                for t in range(NTCH)
            ]
            v_t = [
                apool.tile([128, 4, 128], BF16, name=f"v{t}", tag=f"v{t}")
                for t in range(NTCH)
            ]

            def p_phase(b, px, pxw, pps, vps):
                t0 = b * S
                for tch in range(NTCH):
                    tc0 = t0 + tch * TCH
                    q_ps = [
                        pps.tile([128, TCH], F32, name=f"qps{i}", tag=f"qps{i}")
                        for i in range(QH)
                    ]
                    k_ps = pps.tile([128, TCH], F32, name="kps", tag="kps")
                    v_ps = pps.tile([128, TCH], F32, name="vps0", tag="vps0")
                    for hg in range(HT // 2):
                        x_t = px.tile([128, 2, TCH], F32R, name="xs", tag="xs",
                                      bufs=3)
                        nc.sync.dma_start(
                            x_t[:],
                            xt[hg * 256:(hg + 1) * 256, tc0:tc0 + TCH]
                            .rearrange("(a p) t -> p a t", p=128))
                        wkv_t = pxw.tile([128, 2, 256], F32R, name="wkvs",
                                         tag="wkvs", bufs=3)
                        nc.scalar.dma_start(
                            wkv_t[:],
                            wkv[hg * 256:(hg + 1) * 256, :]
                            .rearrange("(a p) q -> p a q", p=128))
                        for hi in range(2):
                            h = hg * 2 + hi
                            first, last = h == 0, h == HT - 1
                            for qd in range(QH):
                                nc.tensor.matmul(
                                    q_ps[qd][:],
                                    wq_sb[:, h, qd * 128:(qd + 1) * 128],
                                    x_t[:, hi, :], start=first, stop=last)
                            nc.tensor.matmul(k_ps[:], wkv_t[:, hi, 0:128],
                                             x_t[:, hi, :], start=first,
                                             stop=last)
                            nc.tensor.matmul(v_ps[:], wkv_t[:, hi, 128:256],
                                             x_t[:, hi, :], start=first,
                                             stop=last)
                    # RoPE: rotate-half staged via partition-shifted ACT
                    # copies (PSUM frees after one copy pair + the cos mult),
                    # then full-width DVE ops.  tA muls issued first so PSUM
                    # banks release at DVE pace for the next chunk.  On the
                    # last tch, K goes first so attention starts sooner.
                    csl = slice(tch * TCH, (tch + 1) * TCH)
                    units = [(q_ps[qd], qt_t[qd][tch], slice(0, TCH))
                             for qd in range(QH)]
                    units.append((k_ps, kt_t[tch], slice(0, TCH)))
                    if tch == NTCH - 1:
                        units = units[-1:] + units[:-1]
                    vraw = apool.tile([128, TCH], BF16, name="vraw",
                                      tag="vraw", bufs=2)
                    nc.scalar.copy(vraw[:], v_ps[:])
                    sqs, sws = [], []
                    for u, (src, dst, dsl) in enumerate(units):
                        sq = apool.tile([128, TCH], F32, name=f"sq{u}",
                                        tag=f"sq{u}", bufs=1)
                        nc.vector.tensor_copy(sq[:], src[:])
                        sqs.append(sq)
                        sw = apool.tile([128, TCH], F32, name=f"sw{u}",
                                        tag=f"sw{u}", bufs=1)
                        nc.scalar.copy(sw[0:64, :], src[64:128, :])
                        nc.scalar.copy(sw[64:128, :], src[0:64, :])
                        sws.append(sw)
                    for u, (src, dst, dsl) in enumerate(units):
                        tA = apool.tile([128, TCH], F32, name="ropeA",
                                        tag="ropeA", bufs=2)
                        nc.vector.tensor_tensor(
                            tA[:], sqs[u][:], cos_sb[:, csl],
                            mybir.AluOpType.mult)
                        tB = apool.tile([128, TCH], F32, name="ropeB",
                                        tag="ropeB", bufs=2)
                        nc.vector.tensor_tensor(
                            tB[:], sws[u][:], ssin_sb[:, csl],
                            mybir.AluOpType.mult)
                        nc.vector.tensor_tensor(
                            dst[:, dsl], tA[:], tB[:],
                            mybir.AluOpType.add)
                    # V: PE-transpose bf16 staging to token-major (the XBAR
                    # DMA transpose races with its ACT-copy producer on HW)
                    for tt in range(TCH // 128):
                        vt_ps = vps.tile([128, 128], BF16, name="vtp",
                                         tag="vtp")
                        nc.tensor.transpose(
                            vt_ps[:], vraw[:, tt * 128:(tt + 1) * 128],
                            identb_sb[:])
                        nc.scalar.copy(v_t[tch][:, tt, :], vt_ps[:])

            cur = {}

            def w_piece(pend, hch, split_dma=False):
                """o-proj for one 512-col slice of H over pending chunk."""
                pb, pich, pot, _ = pend
                t0 = pb * S + pich * ICH
                ow = apool.tile([128, 4, 512], BF16, name="ow", tag="ow",
                                bufs=2)
                for tt in range(4):
                    w_ps = cur["wps"].tile([128, 512], F32, name="wops",
                                           tag="wops")
                    for od in range(QH):
                        nc.tensor.matmul(
                            w_ps[:],
                            pot[od][:, tt * 128:(tt + 1) * 128],
                            wo_sb[:, od, hch * 512:(hch + 1) * 512],
                            start=(od == 0), stop=(od == QH - 1))
                    nc.vector.tensor_copy(ow[:, tt, :], w_ps[:])
                    if split_dma:
                        nc.sync.dma_start(
                            out[t0 + tt * 128:t0 + (tt + 1) * 128,
                                hch * 512:(hch + 1) * 512],
                            ow[:, tt, :])
                if not split_dma:
                    nc.sync.dma_start(
                        out[t0:t0 + ICH, hch * 512:(hch + 1) * 512]
                        .rearrange("(a p) hh -> p a hh", p=128),
                        ow[:])

            def a_head(b, ich, qh, ot_cur, pend, sched, ap, tpool, norm,
                       sps, ops_, bcps):
                def pop_piece(slot):
                    if pend is not None:
                        for _ in range(sched.get((qh, slot), 0)):
                            if pend[3]:
                                w_piece(pend, pend[3].pop(0))
                isl = slice(ich * ICH, (ich + 1) * ICH)
                p_t = [None] * JT
                st2 = [None]
                o_ps = ops_.tile([128, ICH], F32, name="ops0", tag="ops0")
                t8 = tpool.tile([128, 8, ICH], BF16, name="t8", tag="t8",
                                bufs=1)

                def scores(jt):
                    # scores land in j-tile pairs; one exp per pair halves
                    # the ACT per-instruction overhead
                    if jt % 2 == 0:
                        st2[0] = sps.tile([128, 2, ICH], F32, name="st",
                                          tag="st")
                    nc.tensor.matmul(
                        st2[0][:, jt % 2, :],
                        kt_t[jt // 4][:, (jt % 4) * 128:(jt % 4 + 1) * 128],
                        qt_t[qh][ich][:], start=True, stop=True)
                    if jt % 2 == 1:
                        p2 = ap.tile([128, 2, ICH], BF16, name="ptil",
                                     tag="ptil", bufs=4)
                        nc.scalar.activation(
                            p2[:], st2[0][:],
                            mybir.ActivationFunctionType.Exp, scale=SCALE)
                        p_t[jt - 1] = p2[:, 0, :]
                        p_t[jt] = p2[:, 1, :]
                        nc.gpsimd.tensor_tensor(
                            t8[:, jt // 2, :], p_t[jt - 1], p_t[jt],
                            mybir.AluOpType.add)

                def pv(jt):
                    nc.tensor.matmul(o_ps[:], v_t[jt // 4][:, jt % 4, :],
                                     p_t[jt],
                                     start=(jt == 0), stop=(jt == JT - 1))

                # PE stream: scores run 5 ahead of PV; two o-proj slices of
                # the pending chunk inserted per head (at the head start for
                # qh0, where the boundary stall lives); den/bc before PV tail.
                pop_piece("s")
                for jt in range(5):
                    scores(jt)
                for jt in range(5, 10):
                    pv(jt - 5)
                    scores(jt)
                pop_piece("a")
                for jt in range(10, 12):
                    pv(jt - 5)
                    scores(jt)
                pop_piece("b")
                for jt in range(12, 16):
                    pv(jt - 5)
                    scores(jt)
                pv(11)
                # denominator: bf16 pair-tree tail on Pool, partition-sum on PE
                # last head of the kernel: run the tree tail on DVE (bf16
                # 2x) to shorten the exposed end-of-kernel chain
                teng = nc.vector if (b == B - 1 and ich == NICH - 1
                                     and qh == QH - 1) else nc.gpsimd
                t4 = tpool.tile([128, 4, ICH], BF16, name="t4", tag="t4",
                                bufs=1)
                teng.tensor_tensor(t4[:], t8[:, 0:4, :], t8[:, 4:8, :],
                                   mybir.AluOpType.add)
                t2 = tpool.tile([128, 2, ICH], BF16, name="t2", tag="t2",
                                bufs=1)
                teng.tensor_tensor(t2[:], t4[:, 0:2, :], t4[:, 2:4, :],
                                   mybir.AluOpType.add)
                t1 = tpool.tile([128, ICH], BF16, name="t1", tag="t1", bufs=1)
                teng.tensor_tensor(t1[:], t2[:, 0, :], t2[:, 1, :],
                                   mybir.AluOpType.add)
                # sum-and-broadcast in one matmul: every output partition
                # accumulates the full partition sum of t1
                bc_ps = bcps.tile([128, ICH], F32, name="bc", tag="bc")
                nc.tensor.matmul(bc_ps[:], onesq_sb[:], t1[:],
                                 start=True, stop=True)
                bc_sb = norm.tile([128, ICH], BF16, name="bcs", tag="bcs",
                                  bufs=2)
                nc.vector.reciprocal(bc_sb[:], bc_ps[:])
                for jt in range(12, 16):
                    pv(jt)
                nc.vector.tensor_tensor(ot_cur[qh][:], o_ps[:], bc_sb[:],
                                        mybir.AluOpType.mult)

            pend = None
            for b in range(B):
                with tc.tile_pool(name="px", bufs=1) as px, \
                     tc.tile_pool(name="pxw", bufs=1) as pxw, \
                     tc.tile_pool(name="pps", bufs=1, space="PSUM") as pps, \
                     tc.tile_pool(name="vps", bufs=2, space="PSUM",
                                  side="right") as vps:
                    p_phase(b, px, pxw, pps, vps)
                with tc.tile_pool(name="ap", bufs=1) as ap, \
                     tc.tile_pool(name="tp", bufs=1) as tpool, \
                     tc.tile_pool(name="nrm", bufs=1) as norm, \
                     tc.tile_pool(name="wps", bufs=2, space="PSUM",
                                  side="right") as wps, \
                     tc.tile_pool(name="sps", bufs=2, space="PSUM") as sps, \
                     tc.tile_pool(name="ops", bufs=1, space="PSUM") as ops_, \
                     tc.tile_pool(name="bcp", bufs=1, space="PSUM") as bcps:
                    cur["wps"] = wps
                    for ich in range(NICH):
                        ot_cur = [
                            apool.tile([128, ICH], BF16, name=f"ot{qh}",
                                       tag=f"ot{qh}", bufs=2)
                            for qh in range(QH)
                        ]
                        rem = len(pend[3]) if pend is not None else 0
                        if rem == 6:
                            # post-boundary: front-load qh0 to cover the
                            # projection-pool drain, spread the rest
                            sched = {(0, "s"): 2, (1, "a"): 1, (1, "b"): 1,
                                     (2, "a"): 1, (3, "a"): 1}
                        else:
                            sched = {(0, "s"): 1, (0, "b"): 1,
                                     (1, "a"): 1, (1, "b"): 1,
                                     (2, "a"): 1, (2, "b"): 1,
                                     (3, "a"): 1, (3, "b"): 1}
                        for qh in range(QH):
                            a_head(b, ich, qh, ot_cur, pend, sched, ap,
                                   tpool, norm, sps, ops_, bcps)
                        pend = (b, ich, ot_cur, list(range(NHCH)))
                    if b < B - 1:
                        # two o-proj slices fill the batch-boundary drain
                        for hch in [pend[3].pop(0), pend[3].pop(0)]:
                            w_piece(pend, hch)
                    else:
                        final = list(pend[3])
                        for hch in final:
                            pend[3].remove(hch)
                            w_piece(pend, hch, split_dma=(hch == final[-1]))
    nc.finalize()
    return nc


_NC_CACHE = None


def _get_nc():
    global _NC_CACHE
    if _NC_CACHE is None:
        _NC_CACHE = _build_nc()
    return _NC_CACHE


def _host_tables():
    inv = 1.0 / (ROPE_BASE ** (np.arange(0, HD, 2, dtype=np.float64) / HD))
    t = np.arange(S, dtype=np.float64)
    freqs = np.outer(t, inv)                      # [S, 64]
    emb = np.concatenate([freqs, freqs], axis=1)  # [S, 128]
    cos = np.cos(emb).astype(np.float32).T.copy()   # [128, S]
    sin = np.sin(emb).astype(np.float32).T.copy()
    ssin = sin.copy()
    ssin[0:64, :] *= -1.0
    return np.ascontiguousarray(cos), np.ascontiguousarray(ssin)


def kernel(hidden_states, Wq, Wk, Wv, Wo, trace=False):
    import ml_dtypes
    hs = np.asarray(hidden_states, dtype=np.float32)
    Wq = np.asarray(Wq, dtype=np.float32)
    Wk = np.asarray(Wk, dtype=np.float32)
    Wv = np.asarray(Wv, dtype=np.float32)
    Wo = np.asarray(Wo, dtype=np.float32)

    xt = np.ascontiguousarray(hs.reshape(T, H).T)          # [H, T]
    cos, ssin = _host_tables()
    ones_sq = np.ones((128, 128), dtype=ml_dtypes.bfloat16)

    in_maps = []
    for c in range(NCORES):
        wkv = np.concatenate(
            [Wk[c * HD:(c + 1) * HD, :].T, Wv[c * HD:(c + 1) * HD, :].T],
            axis=1)                                        # [H, 256]
        in_maps.append({
            "xt": xt,
            "wq": np.ascontiguousarray(Wq[c * QD:(c + 1) * QD, :].T),
            "wkv": np.ascontiguousarray(wkv),
            "wob": np.ascontiguousarray(
                Wo[:, c * QD:(c + 1) * QD].T).astype(ml_dtypes.bfloat16),
            "identb": np.eye(128, dtype=ml_dtypes.bfloat16),
            "cosx": cos,
            "ssin": ssin,
            "ones_sq": ones_sq,
        })

    nc = _get_nc()
    res = run_bass_kernel_spmd(nc, in_maps, list(range(NCORES)), trace=trace)
    acc = np.zeros((T, H), dtype=np.float32)
    for c in range(NCORES):
        acc += res.results[c]["out_part"].astype(np.float32)
    out = acc.reshape(B, S, H)
    if trace:
        return out, res
    return out
